# revision 18
# baseline (speedup 1.0000x reference)
"""GroupQueryAttention on 8 trn2 cores.

Sharding: core c = (b, g) with b = c // 4 (batch), g = c % 4 (KV group).
Each core computes the 4 query heads of its group against its batch's
sequence, plus the row-slice of the output projection for those heads.
Host sums the 4 partial outputs per batch (row-parallel Wo) and adds bo.

Per-core layout strategy (everything "transposed", partition dim = the
contraction dim of the next matmul):
  xT   [e=128 x 8, s=2048]   via PE transpose of DMA'd x tiles
  qT   [d=256, s=2048]       = Wq_g^T x^T   (+bq, per-partition add)
  kvT  [d=128, s=2048]       rows 0:64 = k^T, 64:128 = v^T (+bk/bv)
  v_aug[t=128 x 16, 65]      v re-transposed, col 64 = 1.0 (Z column)
  per (head, s-half): for t in 16 tiles:
      scoresT psum [t=128, s=1024] = k_h^T(tile)^T @ q_h^T
      E = exp(0.125 * scoresT)  (ACT, PSUM -> SBUF)
      A@V psum [65, s=1024] += v_aug(t)^T @ E    (row 64 accumulates Z)
  normalize: U^T / Z via reciprocal + PE broadcast of 1/Z over 64 rows
  out^T psum [e=128, s=512] = Wo_g^T slice @ U^T  -> DMA to DRAM [E, S]
"""

import os
import numpy as np
from contextlib import ExitStack

import concourse.bass as bass
import concourse.bacc as bacc
import concourse.mybir as mybir
from concourse.tile import TileContext
from concourse.bass_utils import run_bass_kernel_spmd
from concourse.masks import make_identity

B, S, E = 2, 2048, 1024
H, G, HD = 16, 4, 64
GH = H // G          # heads per group = 4
DG = GH * HD         # q cols per group = 256
N_CORES = 8

FP = mybir.dt.float32
# float32r streams 1 row/cycle (vs 4 for plain fp32) when N >= 256.
MM_FAST = os.environ.get("GQA_MM_FP32R", "1") == "1"
MM_DT = mybir.dt.float32r if MM_FAST else mybir.dt.float32

KE = E // 128        # 8 contraction chunks for projections
NT = S // 128        # 16 t tiles
SC = 512             # matmul moving-dim chunk
NSC = S // SC        # 4
SH = 1024            # s-half for attention psum accumulators
NSH = S // SH        # 2


def mm(x):
    """bitcast an AP for the tensor engine's fast fp32 path"""
    return x.bitcast(MM_DT) if MM_FAST else x


def build_program() -> bass.Bass:
    # Bacc (not plain Bass): its compile() runs move_matmul_waits_to_ldweights
    # + generate_event_semaphores, without which walrus rejects matmuls that
    # accumulated >1 semaphore wait ("Too many sync wait commands").
    nc = bacc.Bacc(None, target_bir_lowering=False)
    x = nc.dram_tensor("xc", [S, E], FP, kind="ExternalInput")
    wq = nc.dram_tensor("wq", [E, DG], FP, kind="ExternalInput")
    wkv = nc.dram_tensor("wkv", [E, 2 * HD], FP, kind="ExternalInput")
    wo = nc.dram_tensor("wo", [DG, E], FP, kind="ExternalInput")
    bq = nc.dram_tensor("bq", [DG], FP, kind="ExternalInput")
    bkv = nc.dram_tensor("bkv", [2 * HD], FP, kind="ExternalInput")
    ot = nc.dram_tensor("ot", [E, S], FP, kind="ExternalOutput")

    with TileContext(nc) as tc, ExitStack() as ctx:
        const = ctx.enter_context(tc.tile_pool(name="const", bufs=1))
        xload = ctx.enter_context(tc.tile_pool(name="xload", bufs=2))
        big = ctx.enter_context(tc.tile_pool(name="big", bufs=1))
        esb_pool = ctx.enter_context(tc.tile_pool(name="esb", bufs=2))
        zpool = ctx.enter_context(tc.tile_pool(name="zpool", bufs=2))
        # PSUM: rot(2 banks) + psc(4 banks) + pav(2 banks) = 8 banks
        rot = ctx.enter_context(tc.tile_pool(name="rot", bufs=2, space="PSUM"))
        pscp = ctx.enter_context(tc.tile_pool(name="pscp", bufs=2, space="PSUM"))
        pavp = ctx.enter_context(tc.tile_pool(name="pavp", bufs=1, space="PSUM"))

        # ---- constants ----
        ident = const.tile([128, 128], FP)
        make_identity(nc, ident)
        # memset cannot emit fp32r (ISA check): memset fp32 scratch, then
        # round through a DVE copy into the matmul-facing ones tiles.
        ones_f = const.tile([128, HD], FP)
        nc.vector.memset(ones_f, 1.0)
        ones_col = const.tile([128, HD], FP)
        nc.vector.tensor_copy(out=mm(ones_col), in_=ones_f)

        # fp32r matmul operands must be written pre-rounded by their
        # producing instruction (BIR verifier rule), and a DMA cannot round:
        # stage each weight load through a scratch tile, rounding via DVE.
        wq_sb = const.tile([128, KE, DG], FP)
        wkv_sb = const.tile([128, KE, 2 * HD], FP)
        wo_sb = const.tile([64, GH, E], FP)
        wq_r = wq.rearrange("(j p) c -> p j c", p=128)
        wo_r = wo.rearrange("(c p) e -> p c e", p=64)
        for dst, src_ap in (
            (wq_sb[:, 0:4, :], wq_r[:, 0:4, :]),
            (wq_sb[:, 4:8, :], wq_r[:, 4:8, :]),
            (wkv_sb[:, :, :], wkv.rearrange("(j p) c -> p j c", p=128)),
            (wo_sb[:, 0:1, :], wo_r[:, 0:1, :]),
            (wo_sb[:, 1:2, :], wo_r[:, 1:2, :]),
            (wo_sb[:, 2:3, :], wo_r[:, 2:3, :]),
            (wo_sb[:, 3:4, :], wo_r[:, 3:4, :]),
        ):
            pdim = dst.shape[0]
            wtmp = xload.tile([128, E], FP, tag="x_sb")
            wview = wtmp[0:pdim, :].rearrange("p (a b) -> p a b", b=dst.shape[-1])
            wview = wview[:, 0 : dst.shape[1], :]
            nc.sync.dma_start(out=wview, in_=src_ap)
            nc.vector.tensor_copy(out=mm(dst), in_=wview)
        bq_sb = const.tile([64, GH], FP)
        nc.sync.dma_start(out=bq_sb, in_=bq.rearrange("(j p) -> p j", p=64))
        bkv_sb = const.tile([128, 1], FP)
        nc.sync.dma_start(out=bkv_sb, in_=bkv.rearrange("(j p) -> p j", p=128))

        # ---- persistent activations ----
        xT = big.tile([128, KE, S], FP)           # 64 KB/part
        qT = big.tile([64, GH, S], FP)            # 32 KB/part on 64 parts
        kvT = big.tile([128, S], FP)              # 8 KB/part
        v_aug = big.tile([128, NT, HD + 2], FP)   # ones | v | ones
        ubarT = big.tile([64, GH, S], FP)         # 32 KB/part on 64 parts

        # ---- phase 1: load x, transpose to xT ----
        for i in range(S // 128):
            x_sb = xload.tile([128, E], FP)
            nc.sync.dma_start(out=x_sb, in_=x[bass.ts(i, 128), :])
            for jb in range(KE // 4):
                pt = rot.tile([128, 512], FP, tag="rot")
                for jj in range(4):
                    j = jb * 4 + jj
                    nc.tensor.transpose(
                        pt[:, bass.ts(jj, 128)], x_sb[:, bass.ts(j, 128)], ident
                    )
                nc.vector.tensor_copy(
                    out=mm(xT[:, bass.ds(jb * 4, 4), bass.ts(i, 128)]),
                    in_=pt.rearrange("p (a b) -> p a b", b=128),
                )

        # ---- phase 2: projections ----
        for sc in range(NSC):
            ssl = bass.ts(sc, SC)
            for h in range(GH):
                pq = rot.tile([128, 512], FP, tag="rot")
                for j in range(KE):
                    nc.tensor.matmul(
                        pq[0:HD, :],
                        mm(wq_sb[:, j, bass.ts(h, HD)]),
                        mm(xT[:, j, ssl]),
                        start=(j == 0),
                        stop=(j == KE - 1),
                    )
                nc.vector.tensor_scalar_add(
                    out=mm(qT[:, h, ssl]), in0=pq[0:HD, :], scalar1=bq_sb[:, h : h + 1]
                )
            pkv = rot.tile([128, 512], FP, tag="rot")
            for j in range(KE):
                nc.tensor.matmul(
                    pkv,
                    mm(wkv_sb[:, j, :]),
                    mm(xT[:, j, ssl]),
                    start=(j == 0),
                    stop=(j == KE - 1),
                )
            nc.vector.tensor_scalar_add(
                out=mm(kvT[:, ssl]), in0=pkv, scalar1=bkv_sb[:, 0:1]
            )

        # ---- phase 2b: v_aug = transpose(vT), ones columns both ends ----
        ones_v = ones_f[:, 0:NT].rearrange("p (a b) -> p a b", b=1)
        nc.vector.tensor_copy(out=mm(v_aug[:, :, 0:1]), in_=ones_v)
        nc.vector.tensor_copy(out=mm(v_aug[:, :, HD + 1 : HD + 2]), in_=ones_v)
        for ib in range(NT // 8):
            pt = rot.tile([128, 512], FP, tag="rot")
            for ii in range(8):
                i = ib * 8 + ii
                nc.tensor.transpose(
                    pt[:, bass.ts(ii, 64)],
                    kvT[HD : 2 * HD, bass.ts(i, 128)],
                    ident[HD : 2 * HD, HD : 2 * HD],
                )
            nc.vector.tensor_copy(
                out=mm(v_aug[:, bass.ds(ib * 8, 8), 1 : HD + 1]),
                in_=pt.rearrange("p (a b) -> p a b", b=HD),
            )

        # ---- phase 3: attention per (head, s-half) ----
        # All heads write A@V to PSUM base 0 (fp32r matmuls require dst
        # base partition 0): U rows 0:63, Z row 64 via the ones column.
        for h in range(GH):
            for sh in range(NSH):
                pav = pavp.tile([128, SH], FP, tag="pav")
                for t in range(NT):
                    psc = pscp.tile([128, SH], FP, tag="psc")
                    for u in range(SH // SC):
                        nc.tensor.matmul(
                            psc[:, bass.ts(u, SC)],
                            mm(kvT[0:HD, bass.ts(t, 128)]),
                            mm(qT[:, h, bass.ds(sh * SH + u * SC, SC)]),
                            start=True,
                            stop=True,
                        )
                    esb = esb_pool.tile([128, SH], FP, tag="esb")
                    nc.scalar.activation(
                        out=mm(esb), in_=psc,
                        func=mybir.ActivationFunctionType.Exp,
                        scale=1.0 / np.sqrt(HD),
                    )
                    for u in range(SH // SC):
                        nc.tensor.matmul(
                            pav[0 : HD + 1, bass.ts(u, SC)],
                            mm(v_aug[:, t, 1 : HD + 2]),
                            mm(esb[:, bass.ts(u, SC)]),
                            start=(t == 0),
                            stop=(t == NT - 1),
                        )
                # stage U rows, compute 1/Z, scale -- all at base 0
                shsl = bass.ds(sh * SH, SH)
                nc.vector.tensor_copy(
                    out=mm(ubarT[:, h, shsl]), in_=pav[0:HD, :]
                )
                zc = zpool.tile([128, SH], FP, tag="zc")
                nc.vector.tensor_copy(
                    out=zc[HD : HD + 1, :], in_=pav[HD : HD + 1, :]
                )
                nc.vector.reciprocal(zc[HD : HD + 1, :], zc[HD : HD + 1, :])
                zrr = zpool.tile([128, SH], FP, tag="zrr")
                nc.vector.tensor_copy(
                    out=mm(zrr[HD : HD + 1, :]), in_=zc[HD : HD + 1, :]
                )
                for u in range(SH // SC):
                    zbt = rot.tile([128, 512], FP, tag="rot")
                    nc.tensor.matmul(
                        zbt[0:HD, :],
                        mm(ones_col[HD : HD + 1, :]),
                        mm(zrr[HD : HD + 1, bass.ts(u, SC)]),
                        start=True,
                        stop=True,
                    )
                    usl = bass.ds(sh * SH + u * SC, SC)
                    nc.vector.tensor_mul(
                        out=mm(ubarT[:, h, usl]),
                        in0=ubarT[:, h, usl],
                        in1=zbt[0:HD, :],
                    )

        # ---- phase 4: output projection (DMA cannot read PSUM: stage) ----
        for sc in range(NSC):
            ssl = bass.ts(sc, SC)
            for et in range(KE):
                po = rot.tile([128, 512], FP, tag="rot")
                for c in range(GH):
                    nc.tensor.matmul(
                        po,
                        mm(wo_sb[:, c, bass.ts(et, 128)]),
                        mm(ubarT[:, c, ssl]),
                        start=(c == 0),
                        stop=(c == GH - 1),
                    )
                ost = xload.tile([128, 512], FP, tag="ost")
                nc.vector.tensor_copy(out=ost, in_=po)
                nc.sync.dma_start(out=ot[bass.ts(et, 128), ssl], in_=ost)

    nc.compile()
    return nc


_prog_cache: dict[str, bass.Bass] = {}


def kernel(x, Wq, bq, Wk, bk, Wv, bv, Wo, bo):
    x = np.ascontiguousarray(np.asarray(x, dtype=np.float32))
    Wq = np.asarray(Wq, dtype=np.float32)
    Wk = np.asarray(Wk, dtype=np.float32)
    Wv = np.asarray(Wv, dtype=np.float32)
    Wo = np.asarray(Wo, dtype=np.float32)
    bq = np.asarray(bq, dtype=np.float32)
    bk = np.asarray(bk, dtype=np.float32)
    bv = np.asarray(bv, dtype=np.float32)
    bo = np.asarray(bo, dtype=np.float32)

    if "nc" not in _prog_cache:
        _prog_cache["nc"] = build_program()
    nc = _prog_cache["nc"]

    in_maps = []
    for c in range(N_CORES):
        b, g = c // G, c % G
        in_maps.append(
            {
                "xc": np.ascontiguousarray(x[b]),
                "wq": np.ascontiguousarray(Wq[:, g * DG : (g + 1) * DG]),
                "wkv": np.ascontiguousarray(
                    np.concatenate(
                        [Wk[:, g * HD : (g + 1) * HD], Wv[:, g * HD : (g + 1) * HD]],
                        axis=1,
                    )
                ),
                "wo": np.ascontiguousarray(Wo[g * DG : (g + 1) * DG, :]),
                "bq": np.ascontiguousarray(bq[g * DG : (g + 1) * DG]),
                "bkv": np.ascontiguousarray(
                    np.concatenate(
                        [bk[g * HD : (g + 1) * HD], bv[g * HD : (g + 1) * HD]]
                    )
                ),
            }
        )

    global _last_in_maps
    _last_in_maps = in_maps
    res = run_bass_kernel_spmd(nc, in_maps, list(range(N_CORES))).results

    out = np.empty((B, S, E), dtype=np.float32)
    for b in range(B):
        acc = res[b * G]["ot"].astype(np.float32)
        for g in range(1, G):
            acc = acc + res[b * G + g]["ot"]
        out[b] = acc.T + bo
    return out
